# revision 1
# baseline (speedup 1.0000x reference)
"""ColorUnpool (gather + segment-max + relu) as an 8-core Trainium2 Bass kernel.

Problem (reference semantics):
    out = zeros([200000, 256]);  out[center_idx] = feat            # centers
    seg = segment_max(feat[edge_src], edge_dst)                    # edges
    out[r] = max(seg[r], 0) for rows r with >= 1 incoming edge

edge_dst only hits rows [50000, 200000), center_idx only [0, 50000), so the
two regions are disjoint.  Sharding: destination rows are split 8 ways;
each core owns 18750 edge-target rows plus 6250 center rows.  The host
builds a padded-CSR (degree-class) layout so that the device kernel is pure
regular tiles:
    per 128-row tile of degree-class d:
        d indirect gathers (feat row per partition) -> d SBUF tiles
        DVE max-reduce chain + clamp-at-0          -> acc tile
        1 indirect scatter of acc to the core's local output rows
Rows with no incoming edge gather a host-appended zero row (relu(0) = 0 ==
the reference's "untouched" value).  Padding slots scatter to a trash row.
"""

import os
import sys
import types

import numpy as np

sys.path.insert(0, "/opt/trn_rl_repo")

N_NODES = 200000
N_CENTERS = 50000
N_EDGES = 400000
FEAT = 256
NCORES = 8
P = 128

R_EDGE = N_NODES - N_CENTERS          # 150000 edge-target rows
RC = R_EDGE // NCORES                 # 18750 edge rows per core
CC = N_CENTERS // NCORES              # 6250 center rows per core
ZROW = N_CENTERS                      # index of the zero row in feat_aug
TRASH = RC                            # local trash row in out_edge

# degree-capacity ladder; extended at runtime if the max degree exceeds it
LADDER = [1, 2, 3, 4, 5, 6, 8, 10, 12, 16, 20, 24, 32, 48, 64, 96, 128]


def _install_profile_hook():
    """Provide antenv.axon_hooks (missing on this image) so that
    run_bass_kernel_spmd(trace=True) can profile via the axon .so."""
    try:
        import antenv
        if "antenv.axon_hooks" in sys.modules:
            return
        from trn_agent_boot.trn_boot import _ntff_profile_via_ctypes
        mod = types.ModuleType("antenv.axon_hooks")
        hook = _ntff_profile_via_ctypes("/opt/axon/libaxon_pjrt.so")
        mod.get_axon_ntff_profile_hook = lambda: hook
        mod.set_axon_ntff_profile_hook = lambda h: None
        sys.modules["antenv.axon_hooks"] = mod
        antenv.axon_hooks = mod
    except Exception:
        pass


def _build_core_plan(rows, srcs, ladder):
    """Host-side CSR/degree-class plan for one core.

    rows: int32 [E_c] local dst row per edge (0..RC-1), unsorted
    srcs: int32 [E_c] feat row per edge
    Returns {cap: (tile_rows [n,], tile_srcs [n, cap])} with n a multiple of
    nothing in particular (padding to tiles of 128 happens later, across
    cores, so tile counts can be equalized).
    """
    order = np.argsort(rows, kind="stable")
    rows_s = rows[order]
    srcs_s = srcs[order]
    deg = np.bincount(rows_s, minlength=RC)

    # capacity class per row (degree-0 rows -> class 1, zero-row source)
    caps = np.asarray(ladder, np.int64)
    cap_idx = np.searchsorted(caps, np.maximum(deg, 1))
    row_cap = caps[cap_idx]                                   # [RC]

    # position of each edge within its row group
    starts = np.concatenate([[0], np.cumsum(deg)[:-1]])       # [RC]
    pos = np.arange(len(rows_s)) - starts[rows_s]             # [E_c]

    plan = {}
    for cap in caps:
        sel = row_cap == cap
        if cap == 1:
            class_rows = np.where(sel)[0].astype(np.int32)    # includes deg-0
        else:
            class_rows = np.where(sel & (deg > 0))[0].astype(np.int32)
        if len(class_rows) == 0:
            continue
        n = len(class_rows)
        local = np.full(RC, -1, np.int64)
        local[class_rows] = np.arange(n)
        # first source per row (repeat-pad keeps the max unchanged);
        # degree-0 rows pad with the zero row
        first = np.full(n, ZROW, np.int32)
        has = deg[class_rows] > 0
        first[has] = srcs_s[starts[class_rows[has]]]
        A = np.repeat(first[:, None], cap, axis=1)            # [n, cap]
        emask = local[rows_s] >= 0
        A[local[rows_s[emask]], pos[emask]] = srcs_s[emask]
        plan[int(cap)] = (class_rows, A)
    return plan


def _build_inputs(feat, center_idx, edge_src, edge_dst):
    """All host preprocessing: returns (in_maps, col_plan, C) where col_plan
    is [(cap, n_tiles, col_base)] shared by all cores."""
    feat = np.ascontiguousarray(np.asarray(feat, np.float32))
    center_idx = np.asarray(center_idx, np.int64)
    edge_src = np.asarray(edge_src, np.int64)
    edge_dst = np.asarray(edge_dst, np.int64)

    feat_aug = np.vstack([feat, np.zeros((1, FEAT), np.float32)])

    # centers: out[center_idx] = feat  (center_idx stays within [0, 50000))
    centr_full = np.zeros((N_CENTERS, FEAT), np.float32)
    centr_full[center_idx] = feat

    local_dst = edge_dst - N_CENTERS
    assert local_dst.min() >= 0 and local_dst.max() < R_EDGE
    core_of = local_dst // RC
    row_of = (local_dst % RC).astype(np.int32)
    src32 = edge_src.astype(np.int32)

    # extend the ladder if needed (deterministic in the inputs)
    max_deg = int(np.bincount(local_dst, minlength=R_EDGE).max())
    ladder = [c for c in LADDER if c <= max(max_deg, 1)]
    if not ladder or ladder[-1] < max_deg:
        ladder.append(max_deg)

    plans = []
    for c in range(NCORES):
        m = core_of == c
        plans.append(_build_core_plan(row_of[m], src32[m], ladder))

    # shared (class, n_tiles) structure: max tile count across cores
    all_caps = sorted({cap for pl in plans for cap in pl})
    col_plan = []
    col = 0
    tiles_of = {}
    for cap in all_caps:
        n_max = max(len(pl[cap][0]) if cap in pl else 0 for pl in plans)
        n_tiles = (n_max + P - 1) // P
        tiles_of[cap] = n_tiles
        col_plan.append((cap, n_tiles, col))
        col += n_tiles * (cap + 1)
    C = col

    in_maps = []
    for c in range(NCORES):
        offs = np.empty((P, C), np.int32)
        for cap, n_tiles, base in col_plan:
            n_slots = n_tiles * P
            if cap in plans[c]:
                class_rows, A = plans[c][cap]
                n = len(class_rows)
            else:
                class_rows = np.empty(0, np.int32)
                A = np.empty((0, cap), np.int32)
                n = 0
            dst = np.full(n_slots, TRASH, np.int32)
            dst[:n] = class_rows
            srcp = np.full((n_slots, cap), ZROW, np.int32)
            srcp[:n] = A
            # tile t, partition p  <->  slot t*P + p
            dst_t = dst.reshape(n_tiles, P)
            src_t = srcp.reshape(n_tiles, P, cap)
            for t in range(n_tiles):
                b = base + t * (cap + 1)
                offs[:, b : b + cap] = src_t[t]
                offs[:, b + cap] = dst_t[t]
        in_maps.append(
            {
                "feat_aug": feat_aug,
                "offs": offs,
                "centr": centr_full[c * CC : (c + 1) * CC],
            }
        )
    return in_maps, col_plan, C


def _build_bass(col_plan, C, bufs=4):
    import concourse.bass as bass
    import concourse.bacc as bacc
    import concourse.mybir as mybir
    import concourse.tile as tile

    nc = bacc.Bacc("TRN2", target_bir_lowering=False, debug=False,
                   num_devices=NCORES)
    t_feat = nc.dram_tensor("feat_aug", [N_CENTERS + 1, FEAT],
                            mybir.dt.float32, kind="ExternalInput")
    t_offs = nc.dram_tensor("offs", [P, C], mybir.dt.int32,
                            kind="ExternalInput")
    t_centr = nc.dram_tensor("centr", [CC, FEAT], mybir.dt.float32,
                             kind="ExternalInput")
    t_oc = nc.dram_tensor("out_center", [CC, FEAT], mybir.dt.float32,
                          kind="ExternalOutput")
    t_oe = nc.dram_tensor("out_edge", [RC + 1, FEAT], mybir.dt.float32,
                          kind="ExternalOutput")

    mx = mybir.AluOpType.max
    with tile.TileContext(nc) as tc:
        with tc.tile_pool(name="sbuf", bufs=bufs) as pool, \
             tc.tile_pool(name="offp", bufs=1) as offp:
            offs = offp.tile([P, C], mybir.dt.int32)
            nc.sync.dma_start(out=offs[:], in_=t_offs[:])
            # center rows: plain DRAM->DRAM copy, separate output tensor
            nc.sync.dma_start(out=t_oc[:], in_=t_centr[:])

            for cap, n_tiles, base in col_plan:
                for t in range(n_tiles):
                    b = base + t * (cap + 1)
                    g = [pool.tile([P, FEAT], mybir.dt.float32,
                                   name=f"g{j}", tag=f"g{j}")
                         for j in range(cap)]
                    acc = pool.tile([P, FEAT], mybir.dt.float32, tag="acc")
                    for j in range(cap):
                        nc.gpsimd.indirect_dma_start(
                            out=g[j][:], out_offset=None, in_=t_feat[:],
                            in_offset=bass.IndirectOffsetOnAxis(
                                ap=offs[:, b + j : b + j + 1], axis=0),
                        )
                    if cap == 1:
                        nc.vector.tensor_scalar_max(acc[:], g[0][:], 0.0)
                    else:
                        nc.vector.tensor_tensor(out=acc[:], in0=g[0][:],
                                                in1=g[1][:], op=mx)
                        for j in range(2, cap):
                            nc.vector.tensor_tensor(out=acc[:], in0=acc[:],
                                                    in1=g[j][:], op=mx)
                        nc.vector.tensor_scalar_max(acc[:], acc[:], 0.0)
                    nc.gpsimd.indirect_dma_start(
                        out=t_oe[:],
                        out_offset=bass.IndirectOffsetOnAxis(
                            ap=offs[:, b + cap : b + cap + 1], axis=0),
                        in_=acc[:], in_offset=None,
                    )
    nc.compile()
    return nc


def kernel(feat, center_idx, edge_src, edge_dst, n_nodes, _trace=False):
    _install_profile_hook()
    import concourse.bass_utils as bass_utils
    bass_utils.upload_artifacts = lambda tmpdir: f"file://{tmpdir}"
    from concourse.bass_utils import run_bass_kernel_spmd

    assert int(n_nodes) == N_NODES

    in_maps, col_plan, C = _build_inputs(feat, center_idx, edge_src, edge_dst)
    nc = _build_bass(col_plan, C)

    kw = {}
    if _trace:
        kw = dict(trace=True)
    res = run_bass_kernel_spmd(nc, in_maps, list(range(NCORES)), **kw)

    out = np.empty((N_NODES, FEAT), np.float32)
    for c in range(NCORES):
        out[c * CC : (c + 1) * CC] = res.results[c]["out_center"]
        out[N_CENTERS + c * RC : N_CENTERS + (c + 1) * RC] = \
            res.results[c]["out_edge"][:RC]
    if _trace:
        return out, res
    return out

